# revision 14
# baseline (speedup 1.0000x reference)
"""Causal self-attention TRN2 Bass kernel (8 NeuronCores).

Sharding: core c handles batch b = c//4 and heads [4*(c%4), 4*(c%4)+4).
Each core computes its heads' QKV projection, causal attention, and the
partial output projection ctx_slice @ w_out_rows; the host sums the 4
partials per batch (exact, since the projection is linear over head
channels) and adds the constant bias terms.

Structure: ALL PSUM goes through one 4-slot ring of [128,1024] tiles
(scores chunks, projection psum, PV output, out-proj psum), giving the
scores pipeline a 2-qt reuse distance past the global-max barrier.
Projection chunks are interleaved with attention groups (group g only
needs Q/K token-groups <= g), so the PE-only projection phase and the
latency-bound softmax pipeline overlap.

Softmax: per 128-row query tile: mask diag in PSUM, per-1024-chunk DVE
max, min-combine, one exp pass per chunk with the global bias, DEFERRED
normalization. P^T comes from the XBAR DMA transpose (blocked
[128, ks, 128] layout; no PE transposes, no PSUM->SBUF P copies). The
row-sum arrives via a ones-column appended to V in the P^T@V matmul
(M=65); 1/s is applied once to the [64,512] context tile.

Phase B (PV + normalize + out-proj) trails phase A by one iteration,
hiding the transpose-DMA latency.

Numerics: matmuls float32r; logits fp32 PSUM, exact row max; P/V/OT/y
bf16 (linear error only).
"""
import math
import os

import numpy as np
import ml_dtypes

import concourse.bacc as bacc
import concourse.bass as bass
import concourse.mybir as mybir
import concourse.tile as tile
from concourse.bass import ds, ts
from concourse.bass_utils import run_bass_kernel_spmd
from concourse.masks import make_identity

# problem shapes (hardcoded per contract)
B, T, C = 2, 2048, 1024
H, D = 16, 64
P = 128
CG = C // P            # 8 contraction tiles over channels
TT = T // P            # 16 token tiles of 128
NG = T // 512          # 4 q-groups of 512
HPAIRS = 2             # head-pairs per core (4 heads/core)
HC = 256               # head channels per core (4 heads * 64)
VW = 65                # V columns per head incl. the ones column
WLAST = [256, 256, 384, 512]   # matmul width of the diagonal part per qt%4
NEG = -1.0e30

F32 = mybir.dt.float32
F32R = mybir.dt.float32r
BF16 = mybir.dt.bfloat16
AX = mybir.AxisListType
OP = mybir.AluOpType
ACTF = mybir.ActivationFunctionType

_CACHE = {}
LAST_RESULT = None


def _build():
    ablate = set(os.environ.get("KERNEL_ABLATE", "").split(","))
    nc = bacc.Bacc("TRN2", target_bir_lowering=False, debug=False, num_devices=8)

    xT_d = nc.dram_tensor("xT", [C, T], F32R, kind="ExternalInput").ap()
    wq_d = nc.dram_tensor("wq", [C, HC], F32R, kind="ExternalInput").ap()
    wk_d = nc.dram_tensor("wk", [C, HC], F32R, kind="ExternalInput").ap()
    wv_d = nc.dram_tensor("wv", [C, HC], F32R, kind="ExternalInput").ap()
    bq_d = nc.dram_tensor("bq", [HC], F32, kind="ExternalInput").ap()
    bk_d = nc.dram_tensor("bk", [HC], F32, kind="ExternalInput").ap()
    wo_d = nc.dram_tensor("wo", [HC, C], BF16, kind="ExternalInput").ap()
    y_d = nc.dram_tensor("y", [T, C], BF16, kind="ExternalOutput").ap()

    with tile.TileContext(nc) as tc:
        with (
            tc.tile_pool(name="const", bufs=1) as const,
            tc.tile_pool(name="big", bufs=1) as big,
            tc.tile_pool(name="ysb", bufs=3) as ysb,
            tc.tile_pool(name="stats", bufs=24) as stats,
            tc.tile_pool(name="rbcp", bufs=2) as rbcp,
            tc.tile_pool(name="psr", bufs=4, space="PSUM") as psr,
        ):
            def pslot():
                return psr.tile([P, 1024], F32, tag="S", name="ps")

            pp_pool = tc.tile_pool(name="pp", bufs=4)
            pp = pp_pool.__enter__()
            pta_pool = tc.tile_pool(name="pta", bufs=8)
            ptap = pta_pool.__enter__()

            # ---- constants / inputs in SBUF (weights first: proj starts
            # as soon as the first xT chunk lands) ----
            ins_pool = tc.tile_pool(name="ins", bufs=1)
            ins = ins_pool.__enter__()
            wq = ins.tile([P, CG, HC], F32R)
            nc.sync.dma_start(wq, wq_d.rearrange("(o p) n -> p o n", p=P))
            wk = ins.tile([P, CG, HC], F32R)
            nc.sync.dma_start(wk, wk_d.rearrange("(o p) n -> p o n", p=P))
            wv = ins.tile([P, CG, HC], F32R)
            nc.sync.dma_start(wv, wv_d.rearrange("(o p) n -> p o n", p=P))
            bq = const.tile([P, HPAIRS], F32)
            nc.sync.dma_start(bq, bq_d.rearrange("(o p) -> p o", p=P))
            bk = const.tile([P, HPAIRS], F32)
            nc.sync.dma_start(bk, bk_d.rearrange("(o p) -> p o", p=P))
            xT = ins.tile([P, CG, T], F32R)
            xTr = xT_d.rearrange("(o p) t -> p o t", p=P)
            for tg in range(NG):
                nc.sync.dma_start(xT[:, :, ts(tg, 512)], xTr[:, :, ts(tg, 512)])
            wo = const.tile([P, HPAIRS, C], BF16)
            nc.sync.dma_start(wo, wo_d.rearrange("(o p) n -> p o n", p=P))

            ident = const.tile([P, P], BF16)
            make_identity(nc, ident)
            # cmask: lower-triangular 0 / -1e30 (keep k <= q)
            cmask = const.tile([P, P], BF16)
            nc.gpsimd.memset(cmask, 0.0)
            nc.gpsimd.affine_select(
                out=cmask,
                in_=cmask,
                compare_op=OP.is_ge,
                fill=NEG,
                base=0,
                pattern=[[-1, P]],
                channel_multiplier=1,
            )

            # ---- persistent intermediates ----
            QT = big.tile([P, HPAIRS, T], F32R)   # head-pair's 2x64 q-rows, *8, +bias
            KT = big.tile([P, HPAIRS, T], F32R)
            VS = big.tile([P, TT, 4 * VW], BF16)  # V rows; per head 64 chans + ones col
            OT = big.tile([P, HPAIRS, T], BF16)   # normalized context^T rows: channels
            if "pv" in ablate or "attn" in ablate:
                nc.vector.memset(OT, 0.0)
            for hh in range(4):
                nc.gpsimd.memset(VS[:, :, VW * hh + 64 : VW * hh + 65], 1.0)

            def emit_proj(hp, tg):
                """Q/K projection for token-group tg of head-pair hp;
                V rows for hp==0 (all heads)."""
                q_ps = pslot()[:, :512]
                for c in range(CG):
                    nc.tensor.matmul(
                        q_ps,
                        wq[:, c, ts(hp, P)],
                        xT[:, c, ts(tg, 512)],
                        start=(c == 0),
                        stop=(c == CG - 1),
                    )
                # QT = psum*8 + 8*bq   (sqrt(D) score scale folded into Q;
                # bq arrives pre-scaled by 8 from the host)
                nc.scalar.activation(
                    QT[:, hp, ts(tg, 512)], q_ps, ACTF.Identity,
                    bias=bq[:, hp : hp + 1], scale=8.0,
                )
                k_ps = pslot()[:, :512]
                for c in range(CG):
                    nc.tensor.matmul(
                        k_ps,
                        wk[:, c, ts(hp, P)],
                        xT[:, c, ts(tg, 512)],
                        start=(c == 0),
                        stop=(c == CG - 1),
                    )
                nc.scalar.activation(
                    KT[:, hp, ts(tg, 512)], k_ps, ACTF.Identity,
                    bias=bk[:, hp : hp + 1], scale=1.0,
                )
                if hp == 0:
                    for tt in range(4 * tg, 4 * tg + 4):
                        v_ps = pslot()[:, :HC]
                        for c in range(CG):
                            nc.tensor.matmul(
                                v_ps,
                                xT[:, c, ts(tt, P)],
                                wv[:, c, :],
                                start=(c == 0),
                                stop=(c == CG - 1),
                            )
                        nc.scalar.copy(
                            VS[:, tt, :].rearrange("p (h e) -> p h e", h=4)[:, :, :64],
                            v_ps.rearrange("p (h e) -> p h e", h=4),
                        )

            def emit_A(hp, h, g):
                """scores -> mask -> chunk maxes -> combine -> exp ->
                XBAR transpose DMA. Returns per-qc (pta_tile, nks)."""
                hrow = 64 * h
                out = []
                for qc in range(4):
                    qt = 4 * g + qc
                    wl = WLAST[qc]
                    wexp = 128 if qc == 0 else wl  # valid width of diag part
                    L = 512 * g + wexp             # valid row length (mult of 128)
                    nks = L // P
                    nch = (L + 1023) // 1024
                    p_t = pp.tile([P, T], BF16, tag="P")
                    mneg = stats.tile([P, 2], F32, tag="mp")
                    ch_tiles = []
                    for ci in range(nch):
                        s_ps = pslot()
                        ch_tiles.append(s_ps)
                        for i in (2 * ci, 2 * ci + 1):
                            if i > g:
                                continue
                            w = wl if i == g else 512
                            off = 512 * (i % 2)
                            diag = i == g
                            nc.tensor.matmul(
                                s_ps[:, ds(off, w)],
                                QT[hrow : hrow + 64, hp, ts(qt, P)],
                                KT[hrow : hrow + 64, hp, ds(512 * i, w)],
                                start=True,
                                stop=not diag,
                                skip_group_check=True,
                            )
                            if diag:
                                # accumulate the causal mask on the PE:
                                # ident^T @ cmask == cmask
                                dof = 512 * (g % 2) + 128 * qc
                                nc.tensor.matmul(
                                    s_ps[:, dof : dof + P],
                                    ident,
                                    cmask,
                                    start=False,
                                    stop=True,
                                    skip_group_check=True,
                                )
                        cw = min(1024, L - 1024 * ci)
                        nc.vector.reduce_max(
                            mneg[:, ci : ci + 1], s_ps[:, :cw], axis=AX.X,
                            negate=True,
                        )
                    if nch > 1:
                        negm = stats.tile([P, 1], F32, tag="negm")
                        nc.gpsimd.tensor_scalar(
                            negm, mneg[:, 0:1], mneg[:, 1:2], None, OP.min
                        )
                    else:
                        negm = mneg[:, 0:1]
                    for ci, s_ps in enumerate(ch_tiles):
                        cw = min(1024, L - 1024 * ci)
                        nc.scalar.activation(
                            p_t[:, ds(1024 * ci, cw)], s_ps[:, :cw],
                            ACTF.Exp, bias=negm, scale=1.0,
                        )
                    pta = ptap.tile([P, TT, P], BF16, tag="pta")
                    nc.sync.dma_start_transpose(pta[:, :nks, :], p_t[:, :L])
                    out.append((pta, nks))
                return out

            def emit_B(hp, h, g, ptas):
                """P^T@[V|1] per qc chain, deferred normalize, out-proj."""
                hh = 2 * hp + h
                hrow = 64 * h
                hcol = VW * hh
                o_ps = pslot()[:VW, :512]
                for qc in range(4):
                    pta, nks = ptas[qc]
                    for ks in range(nks):
                        nc.tensor.matmul(
                            o_ps[:, ts(qc, P)],
                            VS[:, ks, hcol : hcol + VW],
                            pta[:, ks, :],
                            start=(ks == 0),
                            stop=(ks == nks - 1),
                            skip_group_check=True,
                        )
                # deferred normalization: row 64 of o_ps is s^T
                rrow = rbcp.tile([1, 512], F32, tag="rr")
                nc.vector.reciprocal(rrow, o_ps[64:65, :])
                rbc = rbcp.tile([64, 512], F32, tag="rb")
                nc.gpsimd.partition_broadcast(rbc, rrow)
                nc.vector.tensor_tensor(
                    OT[hrow : hrow + 64, hp, ts(g, 512)],
                    o_ps[0:64, :], rbc, OP.mult,
                )
                # output projection once all heads' context for g is ready
                if hp == 1 and h == 1:
                    for tt in range(4 * g, 4 * g + 4):
                        y_ps = pslot()
                        for n in range(2):
                            for hpp in range(HPAIRS):
                                nc.tensor.matmul(
                                    y_ps[:, ts(n, 512)],
                                    OT[:, hpp, ts(tt, P)],
                                    wo[:, hpp, ts(n, 512)],
                                    start=(hpp == 0),
                                    stop=(hpp == HPAIRS - 1),
                                    skip_group_check=True,
                                )
                        y_sb = ysb.tile([P, 1024], BF16, tag="y")
                        if tt % 2 == 0:
                            nc.scalar.copy(y_sb, y_ps)
                        else:
                            nc.vector.tensor_copy(y_sb, y_ps)
                        nc.sync.dma_start(y_d[ts(tt, P), :], y_sb)

            # ---- interleaved projection + attention ----
            do_attn = "attn" not in ablate
            sched = []
            for hp in range(HPAIRS):
                for h in range(2):
                    for g in range(NG):
                        pj = (hp, g) if h == 0 else None
                        sched.append((pj, (hp, h, g) if do_attn else None))
            prev = None
            for pj, it in sched:
                if pj is not None:
                    emit_proj(*pj)
                    if pj == (1, NG - 1):
                        ins_pool.__exit__(None, None, None)
                if it is not None:
                    ptas = emit_A(*it)
                    if prev is not None and "pv" not in ablate:
                        emit_B(*prev[0], prev[1])
                    prev = (it, ptas)
            if prev is not None and "pv" not in ablate:
                emit_B(*prev[0], prev[1])
            if not do_attn:
                ins_pool.__exit__(None, None, None)

            pta_pool.__exit__(None, None, None)
            pp_pool.__exit__(None, None, None)

    nc.compile()
    return nc


def kernel(x, w_qkv, b_qkv, b_out, w_out=None, **kw):
    # tolerate arbitrary kwarg order; reference signature is
    # (x, w_qkv, b_qkv, w_out, b_out)
    if w_out is None:
        w_out = kw.pop("w_out")
    global LAST_RESULT
    x = np.asarray(x, dtype=np.float32)
    w_qkv = np.asarray(w_qkv, dtype=np.float32)
    b_qkv = np.asarray(b_qkv, dtype=np.float32)
    w_out = np.asarray(w_out, dtype=np.float32)
    b_out = np.asarray(b_out, dtype=np.float32)

    if "nc" not in _CACHE:
        _CACHE["nc"] = _build()
    nc = _CACHE["nc"]

    xTs = [np.ascontiguousarray(x[b].T) for b in range(B)]
    in_maps = []
    for c in range(8):
        b = c // 4
        k4 = c % 4
        cols = slice(HC * k4, HC * k4 + HC)
        in_maps.append(
            {
                "xT": xTs[b],
                "wq": np.ascontiguousarray(w_qkv[:, cols]),
                "wk": np.ascontiguousarray(w_qkv[:, C + cols.start : C + cols.stop]),
                "wv": np.ascontiguousarray(
                    w_qkv[:, 2 * C + cols.start : 2 * C + cols.stop]
                ),
                "bq": np.ascontiguousarray(8.0 * b_qkv[cols]),
                "bk": np.ascontiguousarray(b_qkv[C + cols.start : C + cols.stop]),
                "wo": np.ascontiguousarray(w_out[cols, :]).astype(ml_dtypes.bfloat16),
            }
        )

    res = run_bass_kernel_spmd(nc, in_maps, core_ids=list(range(8)))
    LAST_RESULT = res

    y = np.zeros((B, T, C), dtype=np.float32)
    for c in range(8):
        y[c // 4] += np.asarray(res.results[c]["y"]).astype(np.float32)
    # constant terms: V-bias flows through softmax (weights sum to 1) as a
    # constant row shift, so its contribution is exactly b_v @ w_out; plus b_out.
    b_v = b_qkv[2 * C :]
    y += (b_v @ w_out + b_out).astype(np.float32)
    return y


# revision 26
# speedup vs baseline: 1.3697x; 1.3697x over previous
"""Causal self-attention TRN2 Bass kernel (8 NeuronCores).

Sharding: core c handles batch b = c//4 and heads [4*(c%4), 4*(c%4)+4).
Each core computes its heads' QKV projection, causal attention, and the
partial output projection ctx_slice @ w_out_rows; the host sums the 4
partials per batch (exact, since the projection is linear over head
channels) and adds the constant bias terms.

Structure: ALL PSUM goes through one 4-slot ring of [128,1024] tiles
(scores chunks, projection psum, PV output, out-proj psum), giving the
scores pipeline a 2-qt reuse distance past the global-max barrier.
Projection chunks are interleaved with attention groups (group g only
needs Q/K token-groups <= g), so the PE-only projection phase and the
latency-bound softmax pipeline overlap.

Softmax: per 128-row query tile: mask diag in PSUM, per-1024-chunk DVE
max, min-combine, one exp pass per chunk with the global bias, DEFERRED
normalization. P^T comes from the XBAR DMA transpose (blocked
[128, ks, 128] layout; no PE transposes, no PSUM->SBUF P copies). The
row-sum arrives via a ones-column appended to V in the P^T@V matmul
(M=65); 1/s is applied once to the [64,512] context tile.

Phase B (PV + normalize + out-proj) trails phase A by one iteration,
hiding the transpose-DMA latency.

Numerics: matmuls float32r; logits fp32 PSUM, exact row max; P/V/OT/y
bf16 (linear error only).
"""
import math
import os

import numpy as np
import ml_dtypes

import concourse.bacc as bacc
import concourse.bass as bass
import concourse.mybir as mybir
import concourse.tile as tile
from concourse.bass import ds, ts
from concourse.bass_utils import run_bass_kernel_spmd
from concourse.masks import make_identity

# problem shapes (hardcoded per contract)
B, T, C = 2, 2048, 1024
H, D = 16, 64
P = 128
CG = C // P            # 8 contraction tiles over channels
TT = T // P            # 16 token tiles of 128
NG = T // 512          # 4 q-groups of 512
HPAIRS = 2             # head-pairs per core (4 heads/core)
HC = 256               # head channels per core (4 heads * 64)
VW = 65                # V columns per head incl. the ones column
WLAST = [256, 256, 384, 512]   # matmul width of the diagonal part per qt%4
NEG = -1.0e30

F32 = mybir.dt.float32
F32R = mybir.dt.float32r
BF16 = mybir.dt.bfloat16
AX = mybir.AxisListType
OP = mybir.AluOpType
ACTF = mybir.ActivationFunctionType

_CACHE = {}
LAST_RESULT = None
LABELS = {}


def _mk_mark(nc):
    seen = set()

    def mark(label):
        if not os.environ.get("KERNEL_PROF"):
            return
        for b in nc.m.functions[0].blocks:
            for i in b.instructions:
                n = i.name
                if n not in seen:
                    seen.add(n)
                    LABELS[n] = label
    return mark


def _build():
    ablate = set(os.environ.get("KERNEL_ABLATE", "").split(","))
    nc = bacc.Bacc("TRN2", target_bir_lowering=False, debug=False, num_devices=8)
    mark = _mk_mark(nc)

    xT_d = nc.dram_tensor("xT", [C, T], F32R, kind="ExternalInput").ap()
    wq_d = nc.dram_tensor("wq", [C, HC], F32R, kind="ExternalInput").ap()
    wk_d = nc.dram_tensor("wk", [C, HC], F32R, kind="ExternalInput").ap()
    wv_d = nc.dram_tensor("wv", [C, HC], F32R, kind="ExternalInput").ap()
    bq_d = nc.dram_tensor("bq", [HC], F32, kind="ExternalInput").ap()
    bk_d = nc.dram_tensor("bk", [HC], F32, kind="ExternalInput").ap()
    wo_d = nc.dram_tensor("wo", [HC, C], BF16, kind="ExternalInput").ap()
    y_d = nc.dram_tensor("y", [T, C], BF16, kind="ExternalOutput").ap()

    with tile.TileContext(nc) as tc:
        with (
            tc.tile_pool(name="const", bufs=1) as const,
            tc.tile_pool(name="big", bufs=1) as big,
            tc.tile_pool(name="ysb", bufs=2) as ysb,
            tc.tile_pool(name="stats", bufs=24) as stats,
            tc.tile_pool(name="rbcp", bufs=2) as rbcp,
            tc.tile_pool(name="psr", bufs=4, space="PSUM") as psr,
        ):
            def pslot():
                return psr.tile([P, 1024], F32, tag="S", name="ps")

            pp_pool = tc.tile_pool(name="pp", bufs=4)
            pp = pp_pool.__enter__()
            pta_pool = tc.tile_pool(name="pta", bufs=9)
            ptap = pta_pool.__enter__()

            # ---- constants / inputs in SBUF (weights first: proj starts
            # as soon as the first xT chunk lands) ----
            ins_pool = tc.tile_pool(name="ins", bufs=1)
            ins = ins_pool.__enter__()
            wq = ins.tile([P, CG, HC], F32R)
            wqr = wq_d.rearrange("(o p) n -> p o n", p=P)
            for cc in range(2):
                nc.sync.dma_start(wq[:, ds(4 * cc, 4), :], wqr[:, ds(4 * cc, 4), :])
            bq = const.tile([P, HPAIRS], F32)
            nc.sync.dma_start(bq, bq_d.rearrange("(o p) -> p o", p=P))
            bk = const.tile([P, HPAIRS], F32)
            nc.sync.dma_start(bk, bk_d.rearrange("(o p) -> p o", p=P))
            xT = ins.tile([P, CG, T], F32R)
            xTr = xT_d.rearrange("(o p) t -> p o t", p=P)
            for cc in range(2):
                nc.sync.dma_start(
                    xT[:, :, ds(256 * cc, 256)], xTr[:, :, ds(256 * cc, 256)]
                )
            wk = ins.tile([P, CG, HC], F32R)
            wkr = wk_d.rearrange("(o p) n -> p o n", p=P)
            for cc in range(2):
                nc.sync.dma_start(wk[:, ds(4 * cc, 4), :], wkr[:, ds(4 * cc, 4), :])
            wv = ins.tile([P, CG, HC], F32R)
            wvr = wv_d.rearrange("(o p) n -> p o n", p=P)
            for cc in range(2):
                nc.sync.dma_start(wv[:, ds(4 * cc, 4), :], wvr[:, ds(4 * cc, 4), :])
            for tq in range(2, 8):
                nc.sync.dma_start(
                    xT[:, :, ds(256 * tq, 256)], xTr[:, :, ds(256 * tq, 256)]
                )
            wo = const.tile([P, HPAIRS, C], BF16)
            nc.sync.dma_start(wo, wo_d.rearrange("(o p) n -> p o n", p=P))

            ident = const.tile([P, P], BF16)
            make_identity(nc, ident)
            mark("init")
            # cmask: lower-triangular 0 / -1e30 (keep k <= q)
            cmask = const.tile([P, P], BF16)
            nc.gpsimd.memset(cmask, 0.0)
            nc.gpsimd.affine_select(
                out=cmask,
                in_=cmask,
                compare_op=OP.is_ge,
                fill=NEG,
                base=0,
                pattern=[[-1, P]],
                channel_multiplier=1,
            )

            # ---- persistent intermediates ----
            QT = big.tile([P, HPAIRS, T], F32R)   # head-pair's 2x64 q-rows, *8, +bias
            KT = big.tile([P, HPAIRS, T], F32R)
            VS = big.tile([P, TT, 4 * VW], BF16)  # V rows; per head 64 chans + ones col
            OT = big.tile([P, HPAIRS, T], BF16)   # normalized context^T rows: channels
            if "pv" in ablate or "attn" in ablate:
                nc.vector.memset(OT, 0.0)
            for hh in range(4):
                nc.gpsimd.memset(VS[:, :, VW * hh + 64 : VW * hh + 65], 1.0)

            def emit_proj(hp, tg):
                """Q/K projection for token-group tg of head-pair hp;
                V rows for hp==0 (all heads)."""
                q_ps = pslot()[:, :512]
                for c in range(CG):
                    nc.tensor.matmul(
                        q_ps,
                        wq[:, c, ts(hp, P)],
                        xT[:, c, ts(tg, 512)],
                        start=(c == 0),
                        stop=(c == CG - 1),
                    )
                # QT = psum*8 + 8*bq   (sqrt(D) score scale folded into Q;
                # bq arrives pre-scaled by 8 from the host)
                mark("proj.mm")
                nc.scalar.activation(
                    QT[:, hp, ts(tg, 512)], q_ps, ACTF.Identity,
                    bias=bq[:, hp : hp + 1], scale=8.0,
                )
                mark("proj.mv")
                k_ps = pslot()[:, :512]
                for c in range(CG):
                    nc.tensor.matmul(
                        k_ps,
                        wk[:, c, ts(hp, P)],
                        xT[:, c, ts(tg, 512)],
                        start=(c == 0),
                        stop=(c == CG - 1),
                    )
                mark("proj.mm")
                nc.scalar.activation(
                    KT[:, hp, ts(tg, 512)], k_ps, ACTF.Identity,
                    bias=bk[:, hp : hp + 1], scale=1.0,
                )
                mark("proj.mv")
                if hp == 0:
                    for tt in range(4 * tg, 4 * tg + 4):
                        v_ps = pslot()[:, :HC]
                        for c in range(CG):
                            nc.tensor.matmul(
                                v_ps,
                                xT[:, c, ts(tt, P)],
                                wv[:, c, :],
                                start=(c == 0),
                                stop=(c == CG - 1),
                            )
                        mark("proj.mm")
                        nc.scalar.copy(
                            VS[:, tt, :].rearrange("p (h e) -> p h e", h=4)[:, :, :64],
                            v_ps.rearrange("p (h e) -> p h e", h=4),
                        )
                        mark("proj.mv")

            def emit_A_qc(hp, h, g, qc):
                """scores -> mask -> chunk maxes -> combine -> exp ->
                XBAR transpose DMA for one 128-row query tile."""
                hrow = 64 * h
                if True:
                    qt = 4 * g + qc
                    wl = WLAST[qc]
                    wexp = 128 if qc == 0 else wl  # valid width of diag part
                    L = 512 * g + wexp             # valid row length (mult of 128)
                    nks = L // P
                    nch = (L + 1023) // 1024
                    p_t = pp.tile([P, T], BF16, tag="P")
                    mneg = stats.tile([P, 2], F32, tag="mp")
                    ch_tiles = []
                    for ci in range(nch):
                        s_ps = pslot()
                        ch_tiles.append(s_ps)
                        for i in (2 * ci, 2 * ci + 1):
                            if i > g:
                                continue
                            w = wl if i == g else 512
                            off = 512 * (i % 2)
                            diag = i == g
                            nc.tensor.matmul(
                                s_ps[:, ds(off, w)],
                                QT[hrow : hrow + 64, hp, ts(qt, P)],
                                KT[hrow : hrow + 64, hp, ds(512 * i, w)],
                                start=True,
                                stop=not diag,
                                skip_group_check=True,
                            )
                            if diag:
                                # accumulate the causal mask on the PE:
                                # ident^T @ cmask == cmask
                                dof = 512 * (g % 2) + 128 * qc
                                nc.tensor.matmul(
                                    s_ps[:, dof : dof + P],
                                    ident,
                                    cmask,
                                    start=False,
                                    stop=True,
                                    skip_group_check=True,
                                )
                        cw = min(1024, L - 1024 * ci)
                        mark("A.mm")
                        nc.vector.reduce_max(
                            mneg[:, ci : ci + 1], s_ps[:, :cw], axis=AX.X,
                            negate=True,
                        )
                        mark("A.red")
                    if nch > 1:
                        negm = stats.tile([P, 1], F32, tag="negm")
                        nc.gpsimd.tensor_scalar(
                            negm, mneg[:, 0:1], mneg[:, 1:2], None, OP.min
                        )
                    else:
                        negm = mneg[:, 0:1]
                    mark("A.comb")
                    for ci, s_ps in enumerate(ch_tiles):
                        cw = min(1024, L - 1024 * ci)
                        nc.scalar.activation(
                            p_t[:, ds(1024 * ci, cw)], s_ps[:, :cw],
                            ACTF.Exp, bias=negm, scale=1.0,
                        )
                    mark("A.exp")
                    pta = ptap.tile([P, TT, P], BF16, tag="pta")
                    nc.sync.dma_start_transpose(pta[:, :nks, :], p_t[:, :L])
                    mark("A.dma")
                    return (pta, nks)

            def emit_B(hp, h, g, ptas, o_slot):
                """P^T@[V|1] per qc chain, deferred normalize, out-proj."""
                hh = 2 * hp + h
                hrow = 64 * h
                hcol = VW * hh
                o_ps = o_slot[:VW, :512]
                for qc in range(4):
                    pta, nks = ptas[qc]
                    for ks in range(nks):
                        nc.tensor.matmul(
                            o_ps[:, ts(qc, P)],
                            VS[:, ks, hcol : hcol + VW],
                            pta[:, ks, :],
                            start=(ks == 0),
                            stop=(ks == nks - 1),
                            skip_group_check=True,
                        )
                mark("B.pv")
                # deferred normalization: row 64 of o_ps is s^T
                rrow = rbcp.tile([1, 512], F32, tag="rr")
                nc.vector.reciprocal(rrow, o_ps[64:65, :])
                rbc = rbcp.tile([64, 512], F32, tag="rb")
                nc.gpsimd.partition_broadcast(rbc, rrow)
                nc.vector.tensor_tensor(
                    OT[hrow : hrow + 64, hp, ts(g, 512)],
                    o_ps[0:64, :], rbc, OP.mult,
                )
                mark("B.norm")

            def emit_yproj(tt):
                # output projection for one token tile (all heads ready)
                if True:
                    if True:
                        y_ps = pslot()
                        for n in range(2):
                            for hpp in range(HPAIRS):
                                nc.tensor.matmul(
                                    y_ps[:, ts(n, 512)],
                                    OT[:, hpp, ts(tt, P)],
                                    wo[:, hpp, ts(n, 512)],
                                    start=(hpp == 0),
                                    stop=(hpp == HPAIRS - 1),
                                    skip_group_check=True,
                                )
                        y_sb = ysb.tile([P, 1024], BF16, tag="y")
                        if tt % 2 == 0:
                            nc.scalar.copy(y_sb, y_ps)
                        else:
                            nc.vector.tensor_copy(y_sb, y_ps)
                        nc.sync.dma_start(y_d[ts(tt, P), :], y_sb)
                        mark("B.yproj")

            # ---- interleaved projection + attention (B lags A by one) ----
            do_attn = "attn" not in ablate
            if do_attn:
                prev = None
                pend_y = []
                for hp in range(HPAIRS):
                    for h in range(2):
                        for g in range(NG):
                            if h == 0:
                                emit_proj(hp, g)
                                if hp == 1 and g == NG - 1:
                                    ins_pool.__exit__(None, None, None)
                            o_slot = pslot() if prev is not None else None
                            ptas = [emit_A_qc(hp, h, g, qc) for qc in range(4)]
                            if prev is not None and "pv" not in ablate:
                                emit_B(*prev[0], prev[1], o_slot)
                                php, ph, pg = prev[0]
                                if php == 1 and ph == 1:
                                    pend_y.extend(range(4 * pg, 4 * pg + 4))
                                if pend_y:
                                    emit_yproj(pend_y.pop(0))
                            prev = ((hp, h, g), ptas)
                if prev is not None and "pv" not in ablate:
                    emit_B(*prev[0], prev[1], pslot())
                    php, ph, pg = prev[0]
                    if php == 1 and ph == 1:
                        pend_y.extend(range(4 * pg, 4 * pg + 4))
                for tt in pend_y:
                    emit_yproj(tt)
            else:
                for hp in range(HPAIRS):
                    for g in range(NG):
                        emit_proj(hp, g)
                ins_pool.__exit__(None, None, None)

            mark("tail")
            pta_pool.__exit__(None, None, None)
            pp_pool.__exit__(None, None, None)

    nc.compile()
    return nc


def kernel(x, w_qkv, b_qkv, b_out, w_out=None, **kw):
    # tolerate arbitrary kwarg order; reference signature is
    # (x, w_qkv, b_qkv, w_out, b_out)
    if w_out is None:
        w_out = kw.pop("w_out")
    global LAST_RESULT
    x = np.asarray(x, dtype=np.float32)
    w_qkv = np.asarray(w_qkv, dtype=np.float32)
    b_qkv = np.asarray(b_qkv, dtype=np.float32)
    w_out = np.asarray(w_out, dtype=np.float32)
    b_out = np.asarray(b_out, dtype=np.float32)

    if "nc" not in _CACHE:
        _CACHE["nc"] = _build()
    nc = _CACHE["nc"]

    xTs = [np.ascontiguousarray(x[b].T) for b in range(B)]
    in_maps = []
    for c in range(8):
        b = c // 4
        k4 = c % 4
        cols = slice(HC * k4, HC * k4 + HC)
        in_maps.append(
            {
                "xT": xTs[b],
                "wq": np.ascontiguousarray(w_qkv[:, cols]),
                "wk": np.ascontiguousarray(w_qkv[:, C + cols.start : C + cols.stop]),
                "wv": np.ascontiguousarray(
                    w_qkv[:, 2 * C + cols.start : 2 * C + cols.stop]
                ),
                "bq": np.ascontiguousarray(8.0 * b_qkv[cols]),
                "bk": np.ascontiguousarray(b_qkv[C + cols.start : C + cols.stop]),
                "wo": np.ascontiguousarray(w_out[cols, :]).astype(ml_dtypes.bfloat16),
            }
        )

    res = run_bass_kernel_spmd(nc, in_maps, core_ids=list(range(8)))
    LAST_RESULT = res

    y = np.zeros((B, T, C), dtype=np.float32)
    for c in range(8):
        y[c // 4] += np.asarray(res.results[c]["y"]).astype(np.float32)
    # constant terms: V-bias flows through softmax (weights sum to 1) as a
    # constant row shift, so its contribution is exactly b_v @ w_out; plus b_out.
    b_v = b_qkv[2 * C :]
    y += (b_v @ w_out + b_out).astype(np.float32)
    return y
